# revision 1
# baseline (speedup 1.0000x reference)
"""DCT block extractor kernel for 8 TRN2 NeuronCores (pure data parallel).

Math: for each 8x8 block of each [512,512] image, the 2D-DFT bin (u,v) is
  X[u,v] = sum_{r,s} x[r,s] * exp(-2*pi*i*(u*r + v*s)/8)
We need |X| at 6 (u,v) bands, averaged over all 64x64 blocks.

Implementation: contraction over the in-block row index r is done on the
TensorEngine partition axis (block-diagonal weights over 8 row-groups per
64-row chunk); contraction over the in-block column index s is done by PSUM
accumulation across 8 matmuls, each reading a stride-8 column slice of the
image rows. One matmul per (chunk, s):
  lhsT = W[s]  [64, 128]  (k = gi*8+r; Re at m=band*8+gi, Im at m=64+band*8+gi)
  rhs  = rows[:, s::8]    [64, 512]   (free = (img in batch, gj))
Inputs are cast fp32->fp16 by the (gpsimd software-DGE) DMA so the matmul
runs single-pass at 1 cycle/row with fast weight load; PSUM accumulates fp32.
Magnitude via ScalarE Square/Sqrt, accumulate + gj-reduce on VectorE.
Final tiny mean/reshape is done on host from a [48, 24] per-core result.
"""

import os
import sys

import numpy as np

for _p in ("/opt/trn_rl_repo",):
    if os.path.isdir(_p) and _p not in sys.path:
        sys.path.insert(0, _p)

import concourse.bass as bass  # noqa: E402
import concourse.tile as tile  # noqa: E402
from concourse import bacc, mybir  # noqa: E402
from concourse.bass_utils import run_bass_kernel_spmd  # noqa: E402

# Problem shape (hardcoded per contract)
B, C, H, W = 64, 3, 512, 512
N_CORES = 8
BL = B // N_CORES   # 8 batch rows per core
NIMG = BL * C       # 24 images per core (flattened (b, c))
IPB = 8             # images per device-batch
NBATCH = NIMG // IPB  # 3 device-batches
NCHUNK = 8          # 64-row chunks per image
GJ = 64             # block-columns
NFREE = IPB * GJ    # 512 matmul free size
NBANDS = 6

FREQ_BANDS = np.array([[0, 1], [1, 0], [1, 1], [2, 2], [3, 3], [4, 4]]) % 8

BENCH = False          # set True (e.g. from test.py) to profile
BENCH_KWARGS = {}
LAST_EXEC_NS = None
LAST_RESULTS = None

_CACHED_NC = None


def _weights() -> np.ndarray:
    """W[s] in [8, 128, 128] fp16: Re at m=band*8+gi, Im at m=64+band*8+gi.

    Rows 64:128 duplicate rows 0:64 so lhsT can be sliced at base partition
    0 or 64 to match the rhs chunk's base partition."""
    w = np.zeros((8, 64, 128), dtype=np.float32)
    r = np.arange(8)
    for s in range(8):
        for b, (u, v) in enumerate(FREQ_BANDS):
            th = 2.0 * np.pi * (u * r + v * s) / 8.0
            cs, sn = np.cos(th), np.sin(th)
            for gi in range(8):
                w[s, gi * 8 : gi * 8 + 8, b * 8 + gi] = cs
                w[s, gi * 8 : gi * 8 + 8, 64 + b * 8 + gi] = sn
    return np.concatenate([w, w], axis=1).astype(np.float16)


def _build():
    nc = bacc.Bacc("TRN2", target_bir_lowering=False, debug=False, num_devices=N_CORES)
    f32 = mybir.dt.float32
    f16 = mybir.dt.float16

    x_d = nc.dram_tensor("x", [NIMG, H, W], f32, kind="ExternalInput")
    w_d = nc.dram_tensor("w", [8, 128, 128], f16, kind="ExternalInput")
    out_d = nc.dram_tensor("out", [48, NIMG], f32, kind="ExternalOutput")

    with tile.TileContext(nc) as tc:
        with (
            tc.tile_pool(name="consts", bufs=1) as consts,
            tc.tile_pool(name="inp", bufs=12) as inp,
            tc.tile_pool(name="deint", bufs=6) as deint,
            tc.tile_pool(name="psum", bufs=1, space="PSUM") as psum_pool,
            tc.tile_pool(name="work", bufs=3) as work,
            tc.tile_pool(name="accp", bufs=2) as accp,
            tc.tile_pool(name="outp", bufs=2) as outp,
        ):
            w_sb = consts.tile([128, 8, 128], f16)
            nc.sync.dma_start(out=w_sb, in_=w_d[:].transpose([1, 0, 2]))

            # PE warm-up: ~24 dense dummy matmuls (~15us of PE activity) to
            # trip the HAM clock gate to 8/8 (2.4 GHz) before the real work.
            warm = consts.tile([128, 512], f16)
            nc.vector.memset(warm, 0.0)
            ps_w = psum_pool.tile([128, 512], f32, tag="ps0", name="ps_w")
            for i in range(24):
                nc.tensor.matmul(ps_w, warm[:, 0:128], warm, start=(i == 0), stop=(i == 23))

            # DVE does these 8B-granular strided-read copies at 2-4x mode
            # (~0.7us each); ACT is ~2x slower and GpSimd ~10x slower while
            # also blocking DVE via the shared SBUF port lock.
            deint_engines = [nc.vector]
            for bt in range(NBATCH):
                tiles = []
                for t in range(4):  # each tile holds chunks 2t (p 0:64), 2t+1 (p 64:128)
                    it = inp.tile([128, IPB, W], f16)
                    for half in range(2):
                        ch = 2 * t + half
                        # software-DGE DMA casts fp32 -> fp16 in flight
                        nc.gpsimd.dma_start(
                            out=it[64 * half : 64 * half + 64],
                            in_=x_d[
                                bt * IPB : (bt + 1) * IPB, 64 * ch : 64 * ch + 64, :
                            ].transpose([1, 0, 2]),
                        )
                    # pair-deinterleave columns: col gj*8+s -> s_hi*256 + gj*4 + s_lo
                    # (s = 4*s_hi + s_lo) so matmul rhs reads at stride 4 (8 bytes),
                    # below the 16-byte SBUF line-crossing cliff. Reads here are
                    # 4-contiguous-fp16 runs (8B) -> also below the cliff.
                    dt_ = deint.tile([128, IPB, 2, 256], f16)
                    it_v = it.rearrange("p i (g e) -> p i g e", e=8)
                    for s_hi in range(2):
                        eng = deint_engines[(bt * 8 + t * 2 + s_hi) % len(deint_engines)]
                        if eng is nc.scalar:
                            eng.copy(
                                dt_[:, :, s_hi].rearrange("p i (g q) -> p i g q", q=4),
                                it_v[:, :, :, 4 * s_hi : 4 * s_hi + 4],
                            )
                        else:
                            eng.tensor_copy(
                                dt_[:, :, s_hi].rearrange("p i (g q) -> p i g q", q=4),
                                it_v[:, :, :, 4 * s_hi : 4 * s_hi + 4],
                            )
                    tiles.append(dt_)

                acc = accp.tile([48, NFREE], f32)
                pss = [psum_pool.tile([128, NFREE], f32, tag=f"ps{c}", name=f"ps{c}") for c in range(NCHUNK)]
                rhs_vs = [
                    tiles[c // 2][64 * (c % 2) : 64 * (c % 2) + 64].rearrange(
                        "k i h (g q) -> k i h g q", q=4
                    )
                    for c in range(NCHUNK)
                ]
                # s outer / chunk inner: same-base-partition chunks share one
                # LDWEIGHTS per (s, parity) instead of one per matmul
                for s in range(8):
                    for chunk in range(NCHUNK):
                        base = 64 * (chunk % 2)
                        nc.tensor.matmul(
                            pss[chunk],
                            w_sb[base : base + 64, s, :],
                            rhs_vs[chunk][:, :, s // 4, :, s % 4],
                            start=(s == 0),
                            stop=(s == 7),
                        )
                for chunk in range(NCHUNK):
                    ps = pss[chunk]
                    sq_re = work.tile([48, NFREE], f32)
                    sq_im = work.tile([48, NFREE], f32)
                    nc.scalar.square(sq_re, ps[0:48])
                    nc.scalar.square(sq_im, ps[64:112])
                    ss = work.tile([48, NFREE], f32)
                    nc.vector.tensor_add(ss, sq_re, sq_im)
                    if chunk == 0:
                        nc.scalar.sqrt(acc, ss)
                    else:
                        mag = work.tile([48, NFREE], f32)
                        nc.scalar.sqrt(mag, ss)
                        nc.vector.tensor_add(acc, acc, mag)

                ob = outp.tile([48, IPB], f32)
                nc.vector.reduce_sum(
                    out=ob,
                    in_=acc.rearrange("p (i g) -> p i g", g=GJ),
                    axis=mybir.AxisListType.X,
                )
                nc.sync.dma_start(
                    out=out_d[:, bt * IPB : (bt + 1) * IPB], in_=ob
                )

    nc.compile()
    return nc


def kernel(x: np.ndarray) -> np.ndarray:
    global _CACHED_NC, LAST_EXEC_NS, LAST_RESULTS
    x = np.ascontiguousarray(np.asarray(x, dtype=np.float32))
    assert x.shape == (B, C, H, W), x.shape

    if _CACHED_NC is None:
        _CACHED_NC = _build()
    nc = _CACHED_NC

    w = _weights()
    in_maps = [
        {"x": x[i * BL : (i + 1) * BL].reshape(NIMG, H, W), "w": w}
        for i in range(N_CORES)
    ]
    kwargs = dict(BENCH_KWARGS)
    if BENCH:
        kwargs.setdefault("trace", True)
    res = run_bass_kernel_spmd(nc, in_maps, core_ids=list(range(N_CORES)), **kwargs)
    LAST_EXEC_NS = res.exec_time_ns
    LAST_RESULTS = res

    outs = []
    for i in range(N_CORES):
        o = np.asarray(res.results[i]["out"], dtype=np.float64)  # [48, 24]
        o = o.reshape(NBANDS, 8, NIMG)  # [band, gi_l, img]
        o = o.sum(axis=1) / 4096.0      # mean over all 64x64 blocks
        outs.append(o.T.reshape(BL, C * NBANDS))  # img = b_l*C + ch
    return np.concatenate(outs, axis=0).astype(np.float32)



# revision 4
# speedup vs baseline: 1.0187x; 1.0187x over previous
"""DCT block extractor kernel for 8 TRN2 NeuronCores (pure data parallel).

Math: for each 8x8 block of each [512,512] image, the 2D-DFT bin (u,v) is
  X[u,v] = sum_{r,s} x[r,s] * exp(-2*pi*i*(u*r + v*s)/8)
We need |X| at 6 (u,v) bands, averaged over all 64x64 blocks.

v2 design (from microbenchmark evidence):
- Ingest is the wall: SWDGE (gpsimd) DMA casts fp32->fp8e4 in flight.  fp8
  halves the SBUF write side vs fp16 (79us vs 93us for the full 25.2MB/core
  stream) and, critically, lets the matmul read s-slices at stride 8 bytes
  (below the 16B SBUF line-fetch cliff) with NO deinterleave pass: strided
  rhs streams at ~0.88ns/col regardless of PE clock, same rate as the old
  deint+fp16 path, so the whole DVE deint stage and the PE warmup are gone.
- 12 input DMAs (one per [128 rows, 8 img, 512] double-chunk tile), all
  issued up-front (6MB fp8 total fits SBUF), so the SWDGE queue streams
  back-to-back.
- Matmul: contraction over in-block row r on the partition axis
  (block-diagonal fp16 weights, 8 row-groups per 64-row chunk); contraction
  over in-block column s by PSUM accumulation across 8 stride-8 column
  slices.  Chunk pairs (base partition 0/64) run as concurrent row-tiles.
  PSUM layout [96, 512]: Re at band*8+gi (0:48), Im at 48+band*8+gi.
- Magnitude: one ACT square [96,512] per chunk (fp16 out), DVE fp16 adds
  (2x mode), one ACT sqrt [96,512] per chunk pair, DVE fp32 accumulate.
  Per-batch gj-reduction to keep the tail short.
- Final tiny mean/reshape on host from a [96, 24] per-core result.
"""

import os
import sys

import numpy as np

for _p in ("/opt/trn_rl_repo",):
    if os.path.isdir(_p) and _p not in sys.path:
        sys.path.insert(0, _p)

import concourse.bass as bass  # noqa: E402
import concourse.tile as tile  # noqa: E402
from concourse import bacc, mybir  # noqa: E402
from concourse.bass_utils import run_bass_kernel_spmd  # noqa: E402

# Problem shape (hardcoded per contract)
B, C, H, W = 64, 3, 512, 512
N_CORES = 8
BL = B // N_CORES   # 8 batch rows per core
NIMG = BL * C       # 24 images per core (flattened (b, c))
IPB = 8             # images per device-batch
NBATCH = NIMG // IPB  # 3 device-batches
NTILE = 4           # 128-row double-chunk tiles per batch
NBANDS = 6

FREQ_BANDS = np.array([[0, 1], [1, 0], [1, 1], [2, 2], [3, 3], [4, 4]]) % 8

BENCH = False          # set True (e.g. from test.py) to profile
BENCH_KWARGS = {}
LAST_EXEC_NS = None
LAST_RESULTS = None

_CACHED_NC = None


def _weights() -> np.ndarray:
    """W[s] in [8, 128, 112] fp16: Re at m=band*8+gi (0:48), zeros 48:64,
    Im at m=64+band*8+gi (64:112) -- engine partition slices must start at a
    multiple of 32, so the Re/Im halves sit at bases 0 and 64.

    Rows 64:128 duplicate rows 0:64 so lhsT can be sliced at base partition
    0 or 64 to match the rhs chunk's base partition (concurrent row tiles)."""
    w = np.zeros((8, 64, 112), dtype=np.float32)
    r = np.arange(8)
    for s in range(8):
        for b, (u, v) in enumerate(FREQ_BANDS):
            th = 2.0 * np.pi * (u * r + v * s) / 8.0
            cs, sn = np.cos(th), np.sin(th)
            for gi in range(8):
                w[s, gi * 8 : gi * 8 + 8, b * 8 + gi] = cs
                w[s, gi * 8 : gi * 8 + 8, 64 + b * 8 + gi] = sn
    return np.concatenate([w, w], axis=1).astype(np.float16)


def _build():
    nc = bacc.Bacc("TRN2", target_bir_lowering=False, debug=False, num_devices=N_CORES)
    f32 = mybir.dt.float32
    f16 = mybir.dt.float16
    f8 = mybir.dt.float8e4

    x_d = nc.dram_tensor("x", [NIMG, H, W], f32, kind="ExternalInput")
    w_d = nc.dram_tensor("w", [8, 128, 112], f16, kind="ExternalInput")
    out_d = nc.dram_tensor("out", [112, NIMG], f32, kind="ExternalOutput")

    with tile.TileContext(nc) as tc:
        with (
            tc.tile_pool(name="consts", bufs=1) as consts,
            tc.tile_pool(name="inp", bufs=12) as inp,
            tc.tile_pool(name="psum", bufs=1, space="PSUM") as psum_pool,
            tc.tile_pool(name="sq", bufs=4) as sqp,
            tc.tile_pool(name="ssp", bufs=3) as ssp,
            tc.tile_pool(name="mag", bufs=3) as magp,
            tc.tile_pool(name="accp", bufs=1) as accp,
            tc.tile_pool(name="outp", bufs=3) as outp,
        ):
            w_sb = consts.tile([128, 8, 112], f16)
            nc.sync.dma_start(out=w_sb, in_=w_d[:].transpose([1, 0, 2]))

            # all input DMAs up-front: SWDGE streams back-to-back
            tiles = []
            for bt in range(NBATCH):
                for t in range(NTILE):
                    it = inp.tile([128, IPB, W], f8)
                    nc.gpsimd.dma_start(
                        out=it,
                        in_=x_d[
                            bt * IPB : (bt + 1) * IPB,
                            128 * t : 128 * t + 128,
                            :,
                        ].transpose([1, 0, 2]),
                    )
                    tiles.append(it)

            acc = accp.tile([112, NBATCH, W], f32)
            # ss pair tiles, rotated manually; rows 48:64 stay zero
            ss_tiles = []
            for k in range(3):
                sst = consts.tile([112, W], f16, name=f"sst{k}")
                nc.vector.memset(sst, 0.0)
                ss_tiles.append(sst)

            for bt in range(NBATCH):
                for t in range(NTILE):
                    ti = bt * NTILE + t
                    it8 = tiles[ti].rearrange("p i (g e) -> p i g e", e=8)
                    ps_e = psum_pool.tile(
                        [112, W], f32, tag=f"ps{(2 * ti) % 8}", name=f"pse{ti}"
                    )
                    ps_o = psum_pool.tile(
                        [112, W], f32, tag=f"ps{(2 * ti + 1) % 8}", name=f"pso{ti}"
                    )
                    for s in range(8):
                        nc.tensor.matmul(
                            ps_e,
                            w_sb[0:64, s, :],
                            it8[0:64, :, :, s],
                            start=(s == 0),
                            stop=(s == 7),
                        )
                        nc.tensor.matmul(
                            ps_o,
                            w_sb[64:128, s, :],
                            it8[64:128, :, :, s],
                            start=(s == 0),
                            stop=(s == 7),
                        )
                    # magnitude: sq (ACT, fp16 out), add halves (DVE 2x),
                    # sqrt pair (ACT), accumulate (DVE fp32)
                    sq_e = sqp.tile([112, W], f16)
                    sq_o = sqp.tile([112, W], f16)
                    nc.scalar.square(sq_e, ps_e)
                    nc.scalar.square(sq_o, ps_o)
                    # DVE 2-input ops need equal base partitions for SB inputs:
                    # copy the Im half down to base 0 first (fp16 2x mode)
                    tmp_e = sqp.tile([48, W], f16, name=f"tmpe{ti}")
                    tmp_o = sqp.tile([48, W], f16, name=f"tmpo{ti}")
                    nc.vector.tensor_copy(tmp_e, sq_e[64:112])
                    nc.vector.tensor_copy(tmp_o, sq_o[64:112])
                    ss = ss_tiles[ti % 3]
                    nc.vector.tensor_add(ss[0:48], sq_e[0:48], tmp_e)
                    nc.vector.tensor_add(ss[64:112], sq_o[0:48], tmp_o)
                    if t == 0:
                        nc.scalar.sqrt(acc[:, bt, :], ss)
                    else:
                        mag = magp.tile([112, W], f32)
                        nc.scalar.sqrt(mag, ss)
                        nc.vector.tensor_add(acc[:, bt, :], acc[:, bt, :], mag)

                # per-batch gj-reduction -> [96, 8], small out DMA
                ob = outp.tile([112, IPB], f32)
                nc.vector.reduce_sum(
                    out=ob,
                    in_=acc[:, bt, :].rearrange("p (i g) -> p i g", g=64),
                    axis=mybir.AxisListType.X,
                )
                nc.sync.dma_start(
                    out=out_d[:, bt * IPB : (bt + 1) * IPB], in_=ob
                )

    nc.compile()
    return nc


def kernel(x: np.ndarray) -> np.ndarray:
    global _CACHED_NC, LAST_EXEC_NS, LAST_RESULTS
    x = np.ascontiguousarray(np.asarray(x, dtype=np.float32))
    assert x.shape == (B, C, H, W), x.shape

    if _CACHED_NC is None:
        _CACHED_NC = _build()
    nc = _CACHED_NC

    w = _weights()
    in_maps = [
        {"x": x[i * BL : (i + 1) * BL].reshape(NIMG, H, W), "w": w}
        for i in range(N_CORES)
    ]
    kwargs = dict(BENCH_KWARGS)
    if BENCH:
        kwargs.setdefault("trace", True)
    res = run_bass_kernel_spmd(nc, in_maps, core_ids=list(range(N_CORES)), **kwargs)
    LAST_EXEC_NS = res.exec_time_ns
    LAST_RESULTS = res

    outs = []
    for i in range(N_CORES):
        o = np.asarray(res.results[i]["out"], dtype=np.float64)  # [112, 24]
        # rows 0:48 even chunks, 64:112 odd chunks, each band*8+gi; 48:64 zero
        o = o[0:48] + o[64:112]  # [48, 24]
        o = o.reshape(NBANDS, 8, NIMG).sum(axis=1) / 4096.0  # [band, img]
        outs.append(o.T.reshape(BL, C * NBANDS))  # img = b_l*C + ch
    return np.concatenate(outs, axis=0).astype(np.float32)


# revision 5
# speedup vs baseline: 1.1862x; 1.1644x over previous
"""DCT block extractor kernel for 8 TRN2 NeuronCores (pure data parallel).

Math: for each 8x8 block of each [512,512] image, the 2D-DFT bin (u,v) is
  X[u,v] = sum_{r,s} x[r,s] * exp(-2*pi*i*(u*r + v*s)/8)
We need |X| at 6 (u,v) bands, averaged over all 64x64 blocks.

v2 design (from microbenchmark evidence):
- Ingest is the wall: SWDGE (gpsimd) DMA casts fp32->fp8e4 in flight.  fp8
  halves the SBUF write side vs fp16 (79us vs 93us for the full 25.2MB/core
  stream) and, critically, lets the matmul read s-slices at stride 8 bytes
  (below the 16B SBUF line-fetch cliff) with NO deinterleave pass: strided
  rhs streams at ~0.88ns/col regardless of PE clock, same rate as the old
  deint+fp16 path, so the whole DVE deint stage and the PE warmup are gone.
- 12 input DMAs (one per [128 rows, 8 img, 512] double-chunk tile), all
  issued up-front (6MB fp8 total fits SBUF), so the SWDGE queue streams
  back-to-back.
- Matmul: contraction over in-block row r on the partition axis
  (block-diagonal fp16 weights, 8 row-groups per 64-row chunk); contraction
  over in-block column s by PSUM accumulation across 8 stride-8 column
  slices.  Chunk pairs (base partition 0/64) run as concurrent row-tiles.
  PSUM layout [96, 512]: Re at band*8+gi (0:48), Im at 48+band*8+gi.
- Magnitude: one ACT square [96,512] per chunk (fp16 out), DVE fp16 adds
  (2x mode), one ACT sqrt [96,512] per chunk pair, DVE fp32 accumulate.
  Per-batch gj-reduction to keep the tail short.
- Final tiny mean/reshape on host from a [96, 24] per-core result.
"""

import os
import sys

import numpy as np

for _p in ("/opt/trn_rl_repo",):
    if os.path.isdir(_p) and _p not in sys.path:
        sys.path.insert(0, _p)

import concourse.bass as bass  # noqa: E402
import concourse.tile as tile  # noqa: E402
from concourse import bacc, mybir  # noqa: E402
from concourse.bass_utils import run_bass_kernel_spmd  # noqa: E402

# Problem shape (hardcoded per contract)
B, C, H, W = 64, 3, 512, 512
N_CORES = 8
BL = B // N_CORES   # 8 batch rows per core
NIMG = BL * C       # 24 images per core (flattened (b, c))
IPB = 8             # images per device-batch
NBATCH = NIMG // IPB  # 3 device-batches
NTILE = 4           # 128-row double-chunk tiles per batch
NBANDS = 6

FREQ_BANDS = np.array([[0, 1], [1, 0], [1, 1], [2, 2], [3, 3], [4, 4]]) % 8

BENCH = False          # set True (e.g. from test.py) to profile
BENCH_KWARGS = {}
LAST_EXEC_NS = None
LAST_RESULTS = None

_CACHED_NC = None


def _weights() -> np.ndarray:
    """W[s] in [8, 128, 112] fp16: Re at m=band*8+gi (0:48), zeros 48:64,
    Im at m=64+band*8+gi (64:112) -- engine partition slices must start at a
    multiple of 32, so the Re/Im halves sit at bases 0 and 64.

    Rows 64:128 duplicate rows 0:64 so lhsT can be sliced at base partition
    0 or 64 to match the rhs chunk's base partition (concurrent row tiles)."""
    w = np.zeros((8, 64, 112), dtype=np.float32)
    r = np.arange(8)
    for s in range(8):
        for b, (u, v) in enumerate(FREQ_BANDS):
            th = 2.0 * np.pi * (u * r + v * s) / 8.0
            cs, sn = np.cos(th), np.sin(th)
            for gi in range(8):
                w[s, gi * 8 : gi * 8 + 8, b * 8 + gi] = cs
                w[s, gi * 8 : gi * 8 + 8, 64 + b * 8 + gi] = sn
    return np.concatenate([w, w], axis=1).astype(np.float16)


def _build():
    nc = bacc.Bacc("TRN2", target_bir_lowering=False, debug=False, num_devices=N_CORES)
    f32 = mybir.dt.float32
    f16 = mybir.dt.float16
    f8 = mybir.dt.float8e4

    x_d = nc.dram_tensor("x", [NIMG, H, W], f32, kind="ExternalInput")
    w_d = nc.dram_tensor("w", [8, 128, 112], f16, kind="ExternalInput")
    out_d = nc.dram_tensor("out", [112, NIMG], f32, kind="ExternalOutput")

    with tile.TileContext(nc) as tc:
        with (
            tc.tile_pool(name="consts", bufs=1) as consts,
            tc.tile_pool(name="inp", bufs=12) as inp,
            tc.tile_pool(name="psum", bufs=1, space="PSUM") as psum_pool,
            tc.tile_pool(name="sq", bufs=4) as sqp,
            tc.tile_pool(name="ssp", bufs=3) as ssp,
            tc.tile_pool(name="mag", bufs=3) as magp,
            tc.tile_pool(name="accp", bufs=1) as accp,
            tc.tile_pool(name="outp", bufs=3) as outp,
        ):
            # all input DMAs up-front, FIRST in program order: SWDGE streams
            # back-to-back from the earliest possible point.  The last tile is
            # split into two 4-image halves so the end-of-stream compute tail
            # is half as long.
            tiles = []
            for bt in range(NBATCH):
                for t in range(NTILE):
                    last = bt == NBATCH - 1 and t == NTILE - 1
                    if not last:
                        it = inp.tile([128, IPB, W], f8)
                        nc.gpsimd.dma_start(
                            out=it,
                            in_=x_d[
                                bt * IPB : (bt + 1) * IPB,
                                128 * t : 128 * t + 128,
                                :,
                            ].transpose([1, 0, 2]),
                        )
                        tiles.append(it)
                    else:
                        it = inp.tile([128, IPB, W], f8)
                        for hf in range(2):
                            nc.gpsimd.dma_start(
                                out=it[:, 4 * hf : 4 * hf + 4, :],
                                in_=x_d[
                                    bt * IPB + 4 * hf : bt * IPB + 4 * hf + 4,
                                    128 * t : 128 * t + 128,
                                    :,
                                ].transpose([1, 0, 2]),
                            )
                        tiles.append(it)

            w_sb = consts.tile([128, 8, 112], f16)
            nc.sync.dma_start(out=w_sb, in_=w_d[:].transpose([1, 0, 2]))

            acc = accp.tile([112, NBATCH, IPB], f32)
            # ss pair tiles, rotated manually; rows 48:64 stay zero
            ss_tiles = []
            for k in range(3):
                sst = consts.tile([112, W], f16, name=f"sst{k}")
                nc.vector.memset(sst, 0.0)
                ss_tiles.append(sst)

            for bt in range(NBATCH):
                for t in range(NTILE):
                    ti = bt * NTILE + t
                    it8 = tiles[ti].rearrange("p i (g e) -> p i g e", e=8)
                    ps_e = psum_pool.tile(
                        [112, W], f32, tag=f"ps{(2 * ti) % 8}", name=f"pse{ti}"
                    )
                    ps_o = psum_pool.tile(
                        [112, W], f32, tag=f"ps{(2 * ti + 1) % 8}", name=f"pso{ti}"
                    )
                    for s in range(8):
                        nc.tensor.matmul(
                            ps_e,
                            w_sb[0:64, s, :],
                            it8[0:64, :, :, s],
                            start=(s == 0),
                            stop=(s == 7),
                        )
                        nc.tensor.matmul(
                            ps_o,
                            w_sb[64:128, s, :],
                            it8[64:128, :, :, s],
                            start=(s == 0),
                            stop=(s == 7),
                        )
                    # magnitude: sq (ACT, fp16 out), add halves (DVE 2x),
                    # sqrt pair (ACT), accumulate (DVE fp32)
                    sq_e = sqp.tile([112, W], f16)
                    sq_o = sqp.tile([112, W], f16)
                    nc.scalar.square(sq_e, ps_e)
                    nc.scalar.square(sq_o, ps_o)
                    # DVE 2-input ops need equal base partitions for SB inputs:
                    # copy the Im half down to base 0 first (fp16 2x mode)
                    tmp_e = sqp.tile([48, W], f16, name=f"tmpe{ti}")
                    tmp_o = sqp.tile([48, W], f16, name=f"tmpo{ti}")
                    nc.vector.tensor_copy(tmp_e, sq_e[64:112])
                    nc.vector.tensor_copy(tmp_o, sq_o[64:112])
                    ss = ss_tiles[ti % 3]
                    nc.vector.tensor_add(ss[0:48], sq_e[0:48], tmp_e)
                    nc.vector.tensor_add(ss[64:112], sq_o[0:48], tmp_o)
                    mag = magp.tile([112, W], f32)
                    nc.scalar.sqrt(mag, ss)
                    # per-tile gj-reduction keeps the end-of-stream tail short
                    if t == 0:
                        nc.vector.reduce_sum(
                            out=acc[:, bt, :],
                            in_=mag.rearrange("p (i g) -> p i g", g=64),
                            axis=mybir.AxisListType.X,
                        )
                    else:
                        mred = outp.tile([112, IPB], f32)
                        nc.vector.reduce_sum(
                            out=mred,
                            in_=mag.rearrange("p (i g) -> p i g", g=64),
                            axis=mybir.AxisListType.X,
                        )
                        nc.vector.tensor_add(acc[:, bt, :], acc[:, bt, :], mred)

                ob = outp.tile([112, IPB], f32)
                nc.vector.tensor_copy(ob, acc[:, bt, :])
                nc.sync.dma_start(
                    out=out_d[:, bt * IPB : (bt + 1) * IPB], in_=ob
                )

    nc.compile()
    return nc


def kernel(x: np.ndarray) -> np.ndarray:
    global _CACHED_NC, LAST_EXEC_NS, LAST_RESULTS
    x = np.ascontiguousarray(np.asarray(x, dtype=np.float32))
    assert x.shape == (B, C, H, W), x.shape

    if _CACHED_NC is None:
        _CACHED_NC = _build()
    nc = _CACHED_NC

    w = _weights()
    in_maps = [
        {"x": x[i * BL : (i + 1) * BL].reshape(NIMG, H, W), "w": w}
        for i in range(N_CORES)
    ]
    kwargs = dict(BENCH_KWARGS)
    if BENCH:
        kwargs.setdefault("trace", True)
    res = run_bass_kernel_spmd(nc, in_maps, core_ids=list(range(N_CORES)), **kwargs)
    LAST_EXEC_NS = res.exec_time_ns
    LAST_RESULTS = res

    outs = []
    for i in range(N_CORES):
        o = np.asarray(res.results[i]["out"], dtype=np.float64)  # [112, 24]
        # rows 0:48 even chunks, 64:112 odd chunks, each band*8+gi; 48:64 zero
        o = o[0:48] + o[64:112]  # [48, 24]
        o = o.reshape(NBANDS, 8, NIMG).sum(axis=1) / 4096.0  # [band, img]
        outs.append(o.T.reshape(BL, C * NBANDS))  # img = b_l*C + ch
    return np.concatenate(outs, axis=0).astype(np.float32)


# revision 8
# speedup vs baseline: 1.2117x; 1.0215x over previous
"""DCT block extractor kernel for 8 TRN2 NeuronCores (pure data parallel).

Math: for each 8x8 block of each [512,512] image, the 2D-DFT bin (u,v) is
  X[u,v] = sum_{r,s} x[r,s] * exp(-2*pi*i*(u*r + v*s)/8)
We need |X| at 6 (u,v) bands, averaged over all 64x64 blocks.

v2 design (from microbenchmark evidence):
- Ingest is the wall: SWDGE (gpsimd) DMA casts fp32->fp8e4 in flight.  fp8
  halves the SBUF write side vs fp16 (79us vs 93us for the full 25.2MB/core
  stream) and, critically, lets the matmul read s-slices at stride 8 bytes
  (below the 16B SBUF line-fetch cliff) with NO deinterleave pass: strided
  rhs streams at ~0.88ns/col regardless of PE clock, same rate as the old
  deint+fp16 path, so the whole DVE deint stage and the PE warmup are gone.
- 12 input DMAs (one per [128 rows, 8 img, 512] double-chunk tile), all
  issued up-front (6MB fp8 total fits SBUF), so the SWDGE queue streams
  back-to-back.
- Matmul: contraction over in-block row r on the partition axis
  (block-diagonal fp16 weights, 8 row-groups per 64-row chunk); contraction
  over in-block column s by PSUM accumulation across 8 stride-8 column
  slices.  Chunk pairs (base partition 0/64) run as concurrent row-tiles.
  PSUM layout [96, 512]: Re at band*8+gi (0:48), Im at 48+band*8+gi.
- Magnitude: one ACT square [96,512] per chunk (fp16 out), DVE fp16 adds
  (2x mode), one ACT sqrt [96,512] per chunk pair, DVE fp32 accumulate.
  Per-batch gj-reduction to keep the tail short.
- Final tiny mean/reshape on host from a [96, 24] per-core result.
"""

import os
import sys

import numpy as np

for _p in ("/opt/trn_rl_repo",):
    if os.path.isdir(_p) and _p not in sys.path:
        sys.path.insert(0, _p)

import concourse.bass as bass  # noqa: E402
import concourse.tile as tile  # noqa: E402
from concourse import bacc, mybir  # noqa: E402
from concourse.bass_utils import run_bass_kernel_spmd  # noqa: E402

# Problem shape (hardcoded per contract)
B, C, H, W = 64, 3, 512, 512
N_CORES = 8
BL = B // N_CORES   # 8 batch rows per core
NIMG = BL * C       # 24 images per core (flattened (b, c))
IPB = 8             # images per device-batch
NBATCH = NIMG // IPB  # 3 device-batches
NTILE = 4           # 128-row double-chunk tiles per batch
NBANDS = 6

FREQ_BANDS = np.array([[0, 1], [1, 0], [1, 1], [2, 2], [3, 3], [4, 4]]) % 8

BENCH = False          # set True (e.g. from test.py) to profile
BENCH_KWARGS = {}
LAST_EXEC_NS = None
LAST_RESULTS = None

_CACHED_NC = None


def _weights() -> np.ndarray:
    """W[s] in [8, 128, 112] fp16: Re at m=band*8+gi (0:48), zeros 48:64,
    Im at m=64+band*8+gi (64:112) -- engine partition slices must start at a
    multiple of 32, so the Re/Im halves sit at bases 0 and 64.

    Rows 64:128 duplicate rows 0:64 so lhsT can be sliced at base partition
    0 or 64 to match the rhs chunk's base partition (concurrent row tiles)."""
    w = np.zeros((8, 64, 112), dtype=np.float32)
    r = np.arange(8)
    for s in range(8):
        for b, (u, v) in enumerate(FREQ_BANDS):
            th = 2.0 * np.pi * (u * r + v * s) / 8.0
            cs, sn = np.cos(th), np.sin(th)
            for gi in range(8):
                w[s, gi * 8 : gi * 8 + 8, b * 8 + gi] = cs
                w[s, gi * 8 : gi * 8 + 8, 64 + b * 8 + gi] = sn
    return np.concatenate([w, w], axis=1).astype(np.float16)


def _build():
    nc = bacc.Bacc("TRN2", target_bir_lowering=False, debug=False, num_devices=N_CORES)
    f32 = mybir.dt.float32
    f16 = mybir.dt.float16
    f8 = mybir.dt.float8e4

    x_d = nc.dram_tensor("x", [NIMG, H, W], f32, kind="ExternalInput")
    w_d = nc.dram_tensor("w", [8, 128, 112], f16, kind="ExternalInput")
    out_d = nc.dram_tensor("out", [112, NIMG], f32, kind="ExternalOutput")

    with tile.TileContext(nc) as tc:
        with (
            tc.tile_pool(name="consts", bufs=1) as consts,
            tc.tile_pool(name="inp", bufs=12) as inp,
            tc.tile_pool(name="psum", bufs=1, space="PSUM") as psum_pool,
            tc.tile_pool(name="sq", bufs=4) as sqp,
            tc.tile_pool(name="ssp", bufs=3) as ssp,
            tc.tile_pool(name="mag", bufs=3) as magp,
            tc.tile_pool(name="accp", bufs=1) as accp,
            tc.tile_pool(name="outp", bufs=3) as outp,
        ):
            # all input DMAs up-front, FIRST in program order: SWDGE streams
            # back-to-back from the earliest possible point.  The last tile is
            # split into two 4-image halves so the end-of-stream compute tail
            # is half as long.
            tiles = []
            for bt in range(NBATCH):
                for t in range(NTILE):
                    last = bt == NBATCH - 1 and t == NTILE - 1
                    if not last:
                        it = inp.tile([128, IPB, W], f8)
                        nc.gpsimd.dma_start(
                            out=it,
                            in_=x_d[
                                bt * IPB : (bt + 1) * IPB,
                                128 * t : 128 * t + 128,
                                :,
                            ].transpose([1, 0, 2]),
                        )
                        tiles.append(it)
                    else:
                        it = inp.tile([128, IPB, W], f8)
                        for hf in range(2):
                            nc.gpsimd.dma_start(
                                out=it[:, 4 * hf : 4 * hf + 4, :],
                                in_=x_d[
                                    bt * IPB + 4 * hf : bt * IPB + 4 * hf + 4,
                                    128 * t : 128 * t + 128,
                                    :,
                                ].transpose([1, 0, 2]),
                            )
                        tiles.append(it)

            w_sb = consts.tile([128, 8, 112], f16)
            nc.sync.dma_start(out=w_sb, in_=w_d[:].transpose([1, 0, 2]))

            acc = accp.tile([112, NBATCH, IPB], f32)
            # ss pair tiles, rotated manually; rows 48:64 stay zero
            ss_tiles = []
            for k in range(3):
                sst = consts.tile([112, W], f16, name=f"sst{k}")
                nc.vector.memset(sst, 0.0)
                ss_tiles.append(sst)

            def emit_piece(ti, it, i0, i1, tag0, tag1, bt, first):
                """Matmul + magnitude for images [i0:i1) of tile ti."""
                nfree = (i1 - i0) * 64
                it8 = it.rearrange("p i (g e) -> p i g e", e=8)
                ps_e = psum_pool.tile(
                    [112, nfree], f32, tag=f"ps{tag0}", name="pse"
                )
                ps_o = psum_pool.tile(
                    [112, nfree], f32, tag=f"ps{tag1}", name="pso"
                )
                for s in range(8):
                    nc.tensor.matmul(
                        ps_e,
                        w_sb[0:64, s, :],
                        it8[0:64, i0:i1, :, s],
                        start=(s == 0),
                        stop=(s == 7),
                    )
                    nc.tensor.matmul(
                        ps_o,
                        w_sb[64:128, s, :],
                        it8[64:128, i0:i1, :, s],
                        start=(s == 0),
                        stop=(s == 7),
                    )
                # magnitude: sq (ACT, fp16 out), add halves (DVE 2x),
                # sqrt pair (ACT), per-piece gj-reduction (short tail)
                sq_e = sqp.tile([112, nfree], f16, name="sqe")
                sq_o = sqp.tile([112, nfree], f16, name="sqo")
                nc.scalar.square(sq_e, ps_e)
                nc.scalar.square(sq_o, ps_o)
                # DVE 2-input ops need equal base partitions for SB inputs:
                # copy the Im half down to base 0 first (fp16 2x mode)
                tmp_e = sqp.tile([48, nfree], f16, name="tmpe")
                tmp_o = sqp.tile([48, nfree], f16, name="tmpo")
                nc.vector.tensor_copy(tmp_e, sq_e[64:112])
                nc.vector.tensor_copy(tmp_o, sq_o[64:112])
                ss = ss_tiles[ti % 3]
                nc.vector.tensor_add(ss[0:48, 0 : nfree], sq_e[0:48], tmp_e)
                nc.vector.tensor_add(ss[64:112, 0 : nfree], sq_o[0:48], tmp_o)
                mag = magp.tile([112, nfree], f32, name="mag")
                nc.scalar.sqrt(mag, ss[:, 0 : nfree])
                if first:
                    nc.vector.reduce_sum(
                        out=acc[:, bt, i0:i1],
                        in_=mag.rearrange("p (i g) -> p i g", g=64),
                        axis=mybir.AxisListType.X,
                    )
                else:
                    mred = outp.tile([112, i1 - i0], f32, name="mred")
                    nc.vector.reduce_sum(
                        out=mred,
                        in_=mag.rearrange("p (i g) -> p i g", g=64),
                        axis=mybir.AxisListType.X,
                    )
                    nc.vector.tensor_add(
                        acc[:, bt, i0:i1], acc[:, bt, i0:i1], mred
                    )

            for bt in range(NBATCH):
                for t in range(NTILE):
                    ti = bt * NTILE + t
                    last = bt == NBATCH - 1 and t == NTILE - 1
                    if not last:
                        emit_piece(
                            ti, tiles[ti], 0, IPB,
                            (2 * ti) % 8, (2 * ti + 1) % 8, bt, t == 0,
                        )
                    else:
                        # last tile: two 4-image pieces so PE+mag on the first
                        # half overlap the second half's DMA
                        emit_piece(ti, tiles[ti], 0, 4, 6, 7, bt, False)
                        emit_piece(ti, tiles[ti], 4, IPB, 0, 1, bt, False)

                ob = outp.tile([112, IPB], f32)
                nc.vector.tensor_copy(ob, acc[:, bt, :])
                nc.sync.dma_start(
                    out=out_d[:, bt * IPB : (bt + 1) * IPB], in_=ob
                )

    nc.compile()
    return nc


def kernel(x: np.ndarray) -> np.ndarray:
    global _CACHED_NC, LAST_EXEC_NS, LAST_RESULTS
    x = np.ascontiguousarray(np.asarray(x, dtype=np.float32))
    assert x.shape == (B, C, H, W), x.shape

    if _CACHED_NC is None:
        _CACHED_NC = _build()
    nc = _CACHED_NC

    w = _weights()
    in_maps = [
        {"x": x[i * BL : (i + 1) * BL].reshape(NIMG, H, W), "w": w}
        for i in range(N_CORES)
    ]
    kwargs = dict(BENCH_KWARGS)
    if BENCH:
        kwargs.setdefault("trace", True)
    res = run_bass_kernel_spmd(nc, in_maps, core_ids=list(range(N_CORES)), **kwargs)
    LAST_EXEC_NS = res.exec_time_ns
    LAST_RESULTS = res

    outs = []
    for i in range(N_CORES):
        o = np.asarray(res.results[i]["out"], dtype=np.float64)  # [112, 24]
        # rows 0:48 even chunks, 64:112 odd chunks, each band*8+gi; 48:64 zero
        o = o[0:48] + o[64:112]  # [48, 24]
        o = o.reshape(NBANDS, 8, NIMG).sum(axis=1) / 4096.0  # [band, img]
        outs.append(o.T.reshape(BL, C * NBANDS))  # img = b_l*C + ch
    return np.concatenate(outs, axis=0).astype(np.float32)
